# revision 58
# baseline (speedup 1.0000x reference)
"""Trainium2 Bass kernel for DotProductAttention with shared projection.

reference:
    q = query @ W.T + b ; k = key @ W.T + b ; v = value @ W.T + b
    a = q @ k.T ; attn = softmax(a, axis=-1) ; output = attn @ v
    returns (output, attn)

Sharding: 8 cores = 4 batches x 2 target-row halves.  Each core gets its
query row-slice plus the full key/value for its batch, computes its slice
of attn [T, S] and output [T, D].

Math: the scores use the Gram identity
    a = query (W^T W) key^T + (query.W^T b) 1^T + 1 (key.W^T b)^T + |b|^2
which skips the q-projection entirely; G = W^T W is computed once on-chip
from W's natural layout (no transpose needed).  The (query.W^T b) and
|b|^2 terms are constant per softmax row and cancel exactly, so only
ku = key.W^T b is materialized (one K=1 matmul per score block).

Precision: fp16 hi/lo splits (x3 matmuls) along the whole score chain
(G, G.key^T, query.KG) keep the attn error ~2e-4 of scale; v and attn@v
run in single fp16.  fp16 hi/lo works because fp16 products are exact in
the PE's f32 accumulator, so only input-rounding terms survive and the
hi/lo split cancels those to ~2^-22.
"""

import sys
from contextlib import ExitStack

import numpy as np

try:
    import concourse.bass as bass
except ImportError:  # pragma: no cover
    sys.path.insert(0, "/opt/trn_rl_repo")
    import concourse.bass as bass

import concourse.mybir as mybir
import concourse.tile as tile
from concourse import bacc
from concourse.bass_utils import run_bass_kernel_spmd

F32 = mybir.dt.float32
F16 = mybir.dt.float16
P = 128

B_FULL, S_FULL, D_FULL = 4, 4096, 1024
N_CORES = 8
T_FULL = S_FULL * B_FULL // N_CORES  # 2048 target rows per core


def build_nc(T=T_FULL, SL=S_FULL, D=D_FULL, debug_dumps=False):
    """Build the single-core Bass program (SPMD: identical on all cores)."""
    DO = D // P        # d/o outer tiles (8)
    SO = SL // P       # source-row tiles (32)
    TO = T // P        # target-row tiles (16)
    NB = D // 512      # 512-wide free blocks across D (2)
    SB = SL // 512     # 512-wide free blocks across SL (8)

    nc = bacc.Bacc("TRN2", target_bir_lowering=False, debug=False)

    q_in = nc.declare_dram_parameter("query", [T, D], F32, isOutput=False)
    k_in = nc.declare_dram_parameter("key", [SL, D], F32, isOutput=False)
    v_in = nc.declare_dram_parameter("value", [SL, D], F32, isOutput=False)
    w_in = nc.declare_dram_parameter("W", [D, D], F32, isOutput=False)
    b_in = nc.declare_dram_parameter("b", [D], F32, isOutput=False)
    attn_out = nc.declare_dram_parameter("attn", [T, SL], F32, isOutput=True)
    out_out = nc.declare_dram_parameter("out", [T, D], F32, isOutput=True)

    # DRAM scratch
    qh_d = nc.dram_tensor("qh_d", [T, D], F16)
    ql_d = nc.dram_tensor("ql_d", [T, D], F16)
    kh_d = nc.dram_tensor("kh_d", [SL, D], F16)
    kl_d = nc.dram_tensor("kl_d", [SL, D], F16)
    vh_d = nc.dram_tensor("vh_d", [SL, D], F16)
    wh_d = nc.dram_tensor("wh_d", [D, D], F16)
    wl_d = nc.dram_tensor("wl_d", [D, D], F16)
    gh_d = nc.dram_tensor("gh_d", [D, D], F16)     # G = W^T W, fp16 hi
    gl_d = nc.dram_tensor("gl_d", [D, D], F16)     # G lo
    p_d = nc.dram_tensor("p_d", [T, SL], F16)      # unnormalized softmax probs

    dbg = {}
    if debug_dumps:
        for name, shp in [("kth", [P, DO, SL]), ("v16", [SL, D]),
                          ("p", [T, SL]), ("gh", [D, D]), ("gl", [D, D]),
                          ("qh", [T, D]), ("ql", [T, D]),
                          ("ku", [1, SL])]:
            dbg[name] = nc.declare_dram_parameter(
                "dbg_" + name, shp, F16, isOutput=True)

    MM = nc.tensor.matmul
    Exp = mybir.ActivationFunctionType.Exp

    with tile.TileContext(nc) as tc:
        with ExitStack() as ctx:
            consts = ctx.enter_context(tc.tile_pool(name="consts", bufs=1))
            psum = ctx.enter_context(tc.tile_pool(name="psum", bufs=7, space="PSUM"))
            psums = ctx.enter_context(tc.tile_pool(name="psums", bufs=1, space="PSUM"))
            stats = ctx.enter_context(tc.tile_pool(name="stats", bufs=1))

            # ---- constants / small residents ----
            ones = consts.tile([1, 512], F16)
            nc.vector.memset(ones, 1.0)
            b16 = consts.tile([1, D], F16)
            u16 = consts.tile([P, DO], F16)     # u = W^T b, by d-chunks
            recip_z = stats.tile([P, TO], F32)  # softmax 1/rowsum per t-tile
            # ku[s] = key[s].W^T b enters every score row; the other Gram
            # bias terms (query.W^T b and |b|^2) are per-row constants that
            # softmax cancels exactly, so they are never computed.
            ku16 = stats.tile([1, SL], F16)

            # Pools for the projection/Gram phase open BEFORE the phase-0
            # split pools: a pool opened after split closes would reuse
            # split's SBUF addresses and pick up WAR deps on all of phase 0,
            # serializing the kernel start.
            proj_cm = []
            for nm, bufs in (("wnat", 1), ("gout", 4)):
                cm = tc.tile_pool(name=nm, bufs=bufs)
                proj_cm.append((nm, cm, cm.__enter__()))
            pools = {nm: pool for nm, _, pool in proj_cm}
            # W natural-layout fp16 splits, filled during phase 0
            whn = pools["wnat"].tile([P, DO, D], F16, tag="whn")
            wln = pools["wnat"].tile([P, DO, D], F16, tag="wln")

            # ---- phase 0: split inputs into fp16 hi/lo in DRAM ----
            with tc.tile_pool(name="split32", bufs=2) as split_f32, \
                 tc.tile_pool(name="split16", bufs=2) as split_f16:
                b32 = split_f32.tile([1, D], F32, tag="b32")
                nc.sync.dma_start(b32, b_in.rearrange("(a d) -> a d", a=1))
                nc.vector.tensor_copy(b16, b32)
                # W: split straight into the resident natural-layout SBUF
                # tiles (whn/wln feed G and u with no DRAM round trip); the
                # DRAM copies are only for the PV-era transposed loads.
                RC = 512
                C = RC // P
                RCW = 256
                CW = RCW // P
                for i in range(D // RCW):
                    sl_ = slice(i * RCW, (i + 1) * RCW)
                    x = split_f32.tile([P, CW, D], F32, tag="p0w")
                    nc.sync.dma_start(
                        x, w_in[sl_, :].rearrange("(c p) d -> p c d", p=P))
                    hi = whn[:, i * CW:(i + 1) * CW, :]
                    nc.scalar.copy(hi, x)
                    nc.sync.dma_start(
                        wh_d[sl_, :].rearrange("(c p) d -> p c d", p=P), hi)
                    lo = wln[:, i * CW:(i + 1) * CW, :]
                    nc.vector.tensor_sub(lo, x, hi)
                    nc.sync.dma_start(
                        wl_d[sl_, :].rearrange("(c p) d -> p c d", p=P), lo)
                for xsrc, hid, lod, rows, wait_ms in (
                    (k_in, kh_d, kl_d, SL, 0),
                    (q_in, qh_d, ql_d, T, 0.25),
                ):
                    ctx_w = tc.tile_wait_until(wait_ms, enable=wait_ms > 0)
                    ctx_w.__enter__()
                    for i in range(rows // RC):
                        sl_ = slice(i * RC, (i + 1) * RC)
                        x = split_f32.tile([P, C, D], F32, tag="p0x")
                        nc.sync.dma_start(
                            x, xsrc[sl_, :].rearrange("(c p) d -> p c d", p=P))
                        hi = split_f16.tile([P, C, D], F16, tag="p0h")
                        nc.scalar.copy(hi, x)
                        nc.gpsimd.dma_start(
                            hid[sl_, :].rearrange("(c p) d -> p c d", p=P), hi)
                        if lod is not None:
                            lo = split_f16.tile([P, C, D], F16, tag="p0l")
                            nc.vector.tensor_sub(lo, x, hi)
                            nc.gpsimd.dma_start(
                                lod[sl_, :].rearrange("(c p) d -> p c d", p=P), lo)
                    ctx_w.__exit__(None, None, None)

            # ---- G = W^T W, u = W^T b ----
            if True:
                # b transposed onto partitions (tiny), fp16
                b16t = consts.tile([P, DO], F16)
                nc.gpsimd.dma_start(
                    b16t, b_in.rearrange("(po pi) -> pi po", pi=P))

                # u[d] = sum_o W[o, d] b[o], then fp16
                u32 = pools["gout"].tile([P, DO], F32, tag="u32")
                for ai in range(DO):
                    pu_t = psums.tile([P, 512], F32, tag="psmall", name="pu_t")
                    pu = pu_t[:, 0:1]
                    for oi in range(DO):
                        MM(pu, lhsT=whn[:, oi, ai * P:(ai + 1) * P],
                           rhs=b16t[:, oi:oi + 1],
                           start=(oi == 0), stop=(oi == DO - 1))
                    nc.vector.tensor_copy(u32[:, ai:ai + 1], pu)
                nc.vector.tensor_copy(u16, u32)

                # G[a, b] = sum_o W[o, a] W[o, b]  (x3 from hi/lo splits)
                gout = pools["gout"]
                for ai in range(DO):
                    for gb in range(NB):
                        ps = psum.tile([P, 512], F32, tag="ps")
                        for oi in range(DO):
                            MM(ps, lhsT=whn[:, oi, ai * P:(ai + 1) * P],
                               rhs=whn[:, oi, gb * 512:(gb + 1) * 512],
                               start=(oi == 0), stop=False)
                            MM(ps, lhsT=whn[:, oi, ai * P:(ai + 1) * P],
                               rhs=wln[:, oi, gb * 512:(gb + 1) * 512],
                               start=False, stop=False)
                            MM(ps, lhsT=wln[:, oi, ai * P:(ai + 1) * P],
                               rhs=whn[:, oi, gb * 512:(gb + 1) * 512],
                               start=False, stop=(oi == DO - 1))
                        hi = gout.tile([P, 512], F16, tag="ghi")
                        nc.scalar.copy(hi, ps)
                        lo = gout.tile([P, 512], F16, tag="glo")
                        nc.vector.tensor_sub(lo, ps, hi)
                        col = gb * 512
                        nc.sync.dma_start(
                            gh_d[ai * P:(ai + 1) * P, col:col + 512], hi)
                        nc.sync.dma_start(
                            gl_d[ai * P:(ai + 1) * P, col:col + 512], lo)

            for nm, cm, _ in reversed(proj_cm):
                cm.__exit__(None, None, None)

            # ---- KG^T = G @ key^T (resident, hi/lo), ku row, then scores ----
            with tc.tile_pool(name="kres", bufs=1) as kres:
                kth = kres.tile([P, DO, SL], F16)
                ktl = kres.tile([P, DO, SL], F16)

                CK = min(512, SL)
                with tc.tile_pool(name="gload", bufs=1) as gload, \
                     tc.tile_pool(name="kin", bufs=2) as kin:
                    ghh = gload.tile([P, DO, D], F16, tag="ghh")
                    gll = gload.tile([P, DO, D], F16, tag="gll")
                    for di in range(DO):
                        nc.scalar.dma_start(
                            ghh[:, di, :], gh_d[di * P:(di + 1) * P, :])
                        nc.scalar.dma_start(
                            gll[:, di, :], gl_d[di * P:(di + 1) * P, :])
                    for sh in range(SL // CK):
                        khT = kin.tile([P, DO, CK], F16, tag="khT")
                        nc.scalar.dma_start_transpose(
                            khT, kh_d[sh * CK:(sh + 1) * CK, :]
                            .rearrange("s (po pi) -> s po pi", pi=P))
                        klT = kin.tile([P, DO, CK], F16, tag="klT")
                        nc.scalar.dma_start_transpose(
                            klT, kl_d[sh * CK:(sh + 1) * CK, :]
                            .rearrange("s (po pi) -> s po pi", pi=P))
                        for ai in range(DO):
                            for sb in range(CK // 512):
                                ps = psum.tile([P, 512], F32, tag="ps")
                                for di in range(DO):
                                    MM(ps, lhsT=ghh[:, di, ai * P:(ai + 1) * P],
                                       rhs=khT[:, di, sb * 512:(sb + 1) * 512],
                                       start=(di == 0), stop=False)
                                    MM(ps, lhsT=ghh[:, di, ai * P:(ai + 1) * P],
                                       rhs=klT[:, di, sb * 512:(sb + 1) * 512],
                                       start=False, stop=False)
                                    MM(ps, lhsT=gll[:, di, ai * P:(ai + 1) * P],
                                       rhs=khT[:, di, sb * 512:(sb + 1) * 512],
                                       start=False, stop=(di == DO - 1))
                                col = sh * CK + sb * 512
                                nc.scalar.copy(kth[:, ai, col:col + 512], ps)
                                nc.vector.tensor_sub(
                                    ktl[:, ai, col:col + 512], ps,
                                    kth[:, ai, col:col + 512])
                        # ku row for this chunk: ku[s] = key[s] . u + |b|^2
                        for sb in range(CK // 512):
                            pk_t = psums.tile([P, 512], F32, tag="psmall", name="pk_t")
                            pk = pk_t[0:1, :]
                            for di in range(DO):
                                MM(pk, lhsT=u16[:, di:di + 1],
                                   rhs=khT[:, di, sb * 512:(sb + 1) * 512],
                                   start=(di == 0), stop=(di == DO - 1))
                            col = sh * CK + sb * 512
                            nc.vector.tensor_copy(ku16[0:1, col:col + 512], pk)
                    if debug_dumps:
                        nc.sync.dma_start(dbg["kth"][:, :, :], kth)
                        nc.sync.dma_start(dbg["ku"][:, :], ku16)
                        nc.sync.dma_start(dbg["gh"][:, :], gh_d[:, :])
                        nc.sync.dma_start(dbg["gl"][:, :], gl_d[:, :])

                # value hi: cast during DMA early, so the v projection can
                # start the moment the scores phase ends (pure DRAM->DRAM).
                for i in range(2):
                    nc.gpsimd.dma_start(
                        vh_d[i * SL // 2:(i + 1) * SL // 2, :],
                        v_in[i * SL // 2:(i + 1) * SL // 2, :])

                # ---- scores + softmax per target tile ----
                with tc.tile_pool(name="qslice", bufs=2) as qslice, \
                     tc.tile_pool(name="spool", bufs=1) as spool, \
                     tc.tile_pool(name="ppool", bufs=2) as ppool, \
                     tc.tile_pool(name="apool", bufs=1) as apool, \
                     tc.tile_pool(name="small", bufs=8) as small:
                    for ti in range(TO):
                        qsh = qslice.tile([P, DO, P], F16, tag="qsh")
                        nc.scalar.dma_start_transpose(
                            qsh, qh_d[ti * P:(ti + 1) * P, :]
                            .rearrange("t (po pi) -> t po pi", pi=P))
                        qsl = qslice.tile([P, DO, P], F16, tag="qsl")
                        nc.scalar.dma_start_transpose(
                            qsl, ql_d[ti * P:(ti + 1) * P, :]
                            .rearrange("t (po pi) -> t po pi", pi=P))
                        s_sb = spool.tile([P, SL], F32, tag="s_sb")
                        blkmax = small.tile([P, SB], F32, tag="blkmax")
                        for sb in range(SB):
                            ps = psum.tile([P, 512], F32, tag="ps")
                            for ai in range(DO):
                                MM(ps, lhsT=qsh[:, ai, :],
                                   rhs=kth[:, ai, sb * 512:(sb + 1) * 512],
                                   start=(ai == 0), stop=False)
                                MM(ps, lhsT=qsh[:, ai, :],
                                   rhs=ktl[:, ai, sb * 512:(sb + 1) * 512],
                                   start=False, stop=False)
                                MM(ps, lhsT=qsl[:, ai, :],
                                   rhs=kth[:, ai, sb * 512:(sb + 1) * 512],
                                   start=False, stop=False)
                            MM(ps, lhsT=ones[0:1, 0:P],
                               rhs=ku16[0:1, sb * 512:(sb + 1) * 512],
                               start=False, stop=True)
                            # negated per-block max, then f32 copy to SBUF
                            nc.vector.reduce_max(
                                blkmax[:, sb:sb + 1], ps,
                                axis=mybir.AxisListType.X, negate=True)
                            nc.vector.tensor_copy(
                                s_sb[:, sb * 512:(sb + 1) * 512], ps)
                        neg_m = small.tile([P, 1], F32, tag="negm")
                        nc.vector.tensor_reduce(
                            neg_m, blkmax, axis=mybir.AxisListType.X,
                            op=mybir.AluOpType.min)
                        p16 = ppool.tile([P, SL], F16, tag="p16")
                        z = small.tile([P, 1], F32, tag="z")
                        nc.scalar.activation(p16, s_sb, Exp, bias=neg_m,
                                             accum_out=z)
                        nc.sync.dma_start(p_d[ti * P:(ti + 1) * P, :], p16)
                        nc.vector.reciprocal(recip_z[:, ti:ti + 1], z)
                        # normalized attn rows (gpsimd is otherwise idle here)
                        attn_sb = apool.tile([P, SL], F32, tag="attn_sb")
                        nc.gpsimd.tensor_scalar_mul(attn_sb, p16,
                                                    recip_z[:, ti:ti + 1])
                        nc.sync.dma_start(
                            attn_out[ti * P:(ti + 1) * P, :], attn_sb)

            # ---- v projection (deferred here: front window is DMA-bound),
            # ---- then attn @ v per target tile ----
            with tc.tile_pool(name="vres", bufs=1) as vres, \
                 tc.tile_pool(name="wv", bufs=1) as wv, \
                 tc.tile_pool(name="vin", bufs=2) as vin, \
                 tc.tile_pool(name="ptpool", bufs=2) as ptpool, \
                 tc.tile_pool(name="opool", bufs=3) as opool:
                v16 = vres.tile([P, SO, D], F16)
                wht = wv.tile([P, DO, D], F16, tag="wht")
                nc.scalar.dma_start_transpose(
                    wht, wh_d[:, :].rearrange("o (po pi) -> o po pi", pi=P))
                CH = min(512, SL)
                for sh in range(SL // CH):
                    vht = vin.tile([P, DO, CH], F16, tag="vht")
                    nc.scalar.dma_start_transpose(
                        vht, vh_d[sh * CH:(sh + 1) * CH, :]
                        .rearrange("s (po pi) -> s po pi", pi=P))
                    for si in range(CH // P):
                        sg = sh * (CH // P) + si
                        for ob in range(NB):
                            ps = psum.tile([P, 512], F32, tag="ps")
                            for di in range(DO):
                                MM(ps, lhsT=vht[:, di, si * P:(si + 1) * P],
                                   rhs=wht[:, di, ob * 512:(ob + 1) * 512],
                                   start=(di == 0), stop=False)
                            MM(ps, lhsT=ones[0:1, 0:P],
                               rhs=b16[0:1, ob * 512:(ob + 1) * 512],
                               start=False, stop=True)
                            nc.scalar.copy(
                                v16[:, sg, ob * 512:(ob + 1) * 512], ps)
                for ti in range(TO):
                    pt = ptpool.tile([P, SO, P], F16, tag="pt")
                    nc.scalar.dma_start_transpose(
                        pt, p_d[ti * P:(ti + 1) * P, :]
                        .rearrange("t (po pi) -> t po pi", pi=P))
                    out_sb = opool.tile([P, D], F32, tag="out_sb")
                    for ob in range(NB):
                        ps = psum.tile([P, 512], F32, tag="ps")
                        for si in range(SO):
                            MM(ps, lhsT=pt[:, si, :],
                               rhs=v16[:, si, ob * 512:(ob + 1) * 512],
                               start=(si == 0), stop=(si == SO - 1))
                        nc.vector.tensor_scalar_mul(
                            out_sb[:, ob * 512:(ob + 1) * 512], ps,
                            recip_z[:, ti:ti + 1])
                    nc.sync.dma_start(out_out[ti * P:(ti + 1) * P, :], out_sb)

                if debug_dumps:
                    for name, dsrc in [("p", p_d),
                                       ("qh", qh_d), ("ql", ql_d)]:
                        nc.sync.dma_start(dbg[name][:, :], dsrc[:, :])
                    nc.sync.dma_start(
                        dbg["v16"][:, :].rearrange("(so si) o -> si so o", si=P),
                        v16)

    nc.finalize()
    return nc


_NC_CACHE = {}


def _get_nc(T, SL, D):
    key = (T, SL, D)
    if key not in _NC_CACHE:
        _NC_CACHE[key] = build_nc(T, SL, D)
    return _NC_CACHE[key]


def kernel(query, key, value, W, b, _trace=False):
    query = np.ascontiguousarray(np.asarray(query), dtype=np.float32)
    key = np.ascontiguousarray(np.asarray(key), dtype=np.float32)
    value = np.ascontiguousarray(np.asarray(value), dtype=np.float32)
    W = np.ascontiguousarray(np.asarray(W), dtype=np.float32)
    b = np.ascontiguousarray(np.asarray(b), dtype=np.float32)

    B, S, D = query.shape
    n_cores = N_CORES
    halves = n_cores // B
    T = S // halves

    nc = _get_nc(T, S, D)
    in_maps = []
    for c in range(n_cores):
        bb, h = divmod(c, halves)
        in_maps.append({
            "query": np.ascontiguousarray(query[bb, h * T:(h + 1) * T, :]),
            "key": key[bb],
            "value": value[bb],
            "W": W,
            "b": b,
        })
    res = run_bass_kernel_spmd(nc, in_maps, core_ids=list(range(n_cores)),
                               trace=_trace)
    attn = np.empty((B, S, S), np.float32)
    out = np.empty((B, S, D), np.float32)
    for c, r in enumerate(res.results):
        bb, h = divmod(c, halves)
        attn[bb, h * T:(h + 1) * T, :] = r["attn"]
        out[bb, h * T:(h + 1) * T, :] = r["out"]
    if _trace:
        return (out, attn), res
    return (out, attn)
